# revision 5
# baseline (speedup 1.0000x reference)
"""Trainium2 Bass kernel for nn_DetectionLoss (B=16, N=25000, M=64).

V5: host-side area-sort of preds (loss is permutation-invariant over
preds). Each SBUF partition row then holds 196 area-consecutive preds,
so 1/(ap+at) is ~constant per (partition, GT) and the whole coarse
normalization moves out of the N*M bulk:
  bulk/group: 4 minmax + 2 sub + relu(ACT) + inter-mult + macc  (f16 2x)
  per image:  m1 = reduce(macc); m1q = m1 * recip(ap_mid + at)  ([P,64])
Engines: Pool {lty, rby (+ltx 1/7)}, DVE the rest, ACT relu+bcasts.
Single act table (natural_log_exp) for focal exp/ln + relu + copies.
Tail: full-row refine argmax, early g5 gather, DVE-only dedup chain.
"""

import numpy as np

B, N, M = 16, 25000, 64
P = 128
SLOTS = 196
IMGS_PER_CORE = 2
N_CORES = 8
UG = 28
NG = SLOTS // UG   # 7
K = 3
POOL_LTX_GROUPS = (3,)

PAD_PART = 127
PAD_START = N - PAD_PART * SLOTS   # 108

_cache = {}


def _build(debug_dumps=False):
    import concourse.bass as bass
    import concourse.bacc as bacc
    import concourse.mybir as mybir
    from concourse import tile
    from concourse.bass import IndirectOffsetOnAxis
    from concourse.masks import make_identity

    f32 = mybir.dt.float32
    f16 = mybir.dt.float16
    u32 = mybir.dt.uint32
    i32 = mybir.dt.int32
    Alu = mybir.AluOpType
    Act = mybir.ActivationFunctionType
    X = mybir.AxisListType.X

    nc = bacc.Bacc("TRN2", target_bir_lowering=False, debug=False,
                   num_devices=N_CORES)

    preds_d = nc.dram_tensor("preds", [IMGS_PER_CORE, N, 5], f32, kind="ExternalInput")
    targets_d = nc.dram_tensor("targets", [IMGS_PER_CORE, M, 4], f32, kind="ExternalInput")
    sums_out_d = nc.dram_tensor("sums_out", [2, 3], f32, kind="ExternalOutput")
    fsum_out_d = nc.dram_tensor("fsum_out", [2, 1], f32, kind="ExternalOutput")
    drvI_d = nc.dram_tensor("drv_scratch", [IMGS_PER_CORE * P, SLOTS * 5], f32)

    EPS = np.float32(1e-7)
    C_4PI2 = np.float32(4.0 / (np.pi ** 2))
    SP_SEED = [0.041064513, -0.156028432, 0.304672365, -0.496368282, 0.999887926]
    AT_POLY = [0.0030496317, -0.0168262157, 0.0438537714, -0.0759666934,
               0.1068136135, -0.1421318243, 0.1999371457, -0.3333312071,
               0.9999999881]

    with tile.TileContext(nc) as tc:
        with (
            tc.tile_pool(name="cst", bufs=1) as cst,
            tc.tile_pool(name="pim", bufs=2) as pim,      # per-image tiles
            tc.tile_pool(name="grp", bufs=4) as grp,      # bulk group temps
            tc.tile_pool(name="ref", bufs=1) as ref,      # refine/tail tiles
            tc.tile_pool(name="sml", bufs=2) as sml,      # small temps
            tc.tile_pool(name="psum", bufs=1,
                         space=bass.MemorySpace.PSUM) as psum,
            tc.tile_pool(name="psum2", bufs=2,
                         space=bass.MemorySpace.PSUM) as psum2,
        ):
            lp = nc.allow_low_precision(reason="fp16 coarse scoring pass")
            lp.__enter__()

            def dbg(name, ap, shape, dtype=f32):
                if not debug_dumps:
                    return
                t = nc.dram_tensor(f"dbg_{name}", shape, dtype, kind="ExternalOutput")
                nc.sync.dma_start(t.ap(), ap)

            # ---------------- constants ----------------
            iota_p = cst.tile([P, 1], i32, tag="iota_p")
            nc.gpsimd.iota(iota_p[:], pattern=[[1, 1]], base=0, channel_multiplier=1)
            iota_f = cst.tile([P, P], i32, tag="iota_f")
            nc.gpsimd.iota(iota_f[:], pattern=[[1, P]], base=0, channel_multiplier=0)
            ident = cst.tile([P, P], f32, tag="ident")
            make_identity(nc, ident[:])
            ones_row = cst.tile([1, P], f32, tag="ones_row")
            nc.gpsimd.memset(ones_row[:], 1.0)
            ones_col = cst.tile([P, 1], f32, tag="ones_col")
            nc.gpsimd.memset(ones_col[:], 1.0)
            iota_pf = cst.tile([P, 1], f32, tag="iota_pf")
            nc.vector.tensor_copy(iota_pf[:], iota_p[:])
            iota_ff = cst.tile([P, P], f32, tag="iota_ff")
            nc.vector.tensor_copy(iota_ff[:], iota_f[:])
            iota_kf = cst.tile([P, 8], f32, tag="iota_kf")
            nc.vector.tensor_copy(iota_kf[:], iota_f[:, :8])
            sameimg = cst.tile([P, P], f32, tag="sameimg")
            halfp = cst.tile([P, 1], f32, tag="halfp")
            nc.vector.tensor_scalar(halfp[:], iota_pf[:], float(M), None, op0=Alu.is_ge)
            nc.vector.tensor_scalar(sameimg[:], iota_ff[:], float(M), None, op0=Alu.is_ge)
            nc.vector.tensor_scalar(sameimg[:], sameimg[:], halfp[:], None, op0=Alu.is_equal)
            ltmask = cst.tile([P, P], f32, tag="ltmask")
            nc.vector.tensor_scalar(ltmask[:], iota_ff[:], iota_pf[:], None, op0=Alu.is_lt)
            nc.vector.tensor_tensor(ltmask[:], ltmask[:], sameimg[:], op=Alu.mult)
            imgoffPf = cst.tile([P, 1], f32, tag="imgoffPf")
            nc.vector.tensor_scalar(imgoffPf[:], halfp[:], float(P), None, op0=Alu.mult)
            imgoffNf = cst.tile([P, 1], f32, tag="imgoffNf")
            nc.vector.tensor_scalar(imgoffNf[:], halfp[:], float(N), None, op0=Alu.mult)
            halfind = cst.tile([P, 2], f32, tag="halfind")
            nc.vector.tensor_scalar(halfind[:, 1:2], halfp[:], 1.0, None, op0=Alu.mult)
            nc.vector.tensor_scalar(halfind[:, 0:1], halfp[:], -1.0, 1.0,
                                    op0=Alu.mult, op1=Alu.add)

            mall = ref.tile([P, P], f32, tag="mall")
            conf16 = ref.tile([P, IMGS_PER_CORE, SLOTS], f16, tag="conf16")

            # ================= prep: both images =================
            imgs = []
            for b in range(IMGS_PER_CORE):
                predsI = pim.tile([P, SLOTS, 5], f32, tag="predsI")
                src = preds_d.ap()[b].rearrange("n c -> (n c)")
                # pads live only in partition 127, slots PAD_START:, but the
                # engine needs a 32-aligned partition base; the preds DMA then
                # overwrites rows 96..126 with real data
                nc.gpsimd.memset(predsI[96:, PAD_START:, 0:2], 50.0)
                nc.gpsimd.memset(predsI[96:, PAD_START:, 2:4], 1e-4)
                nc.gpsimd.memset(predsI[96:, PAD_START:, 4:5], -80.0)
                nc.sync.dma_start(
                    predsI[:PAD_PART],
                    src[: PAD_PART * SLOTS * 5].rearrange("(p f) -> p f", p=PAD_PART)
                    .rearrange("p (s c) -> p s c", c=5))
                nc.sync.dma_start(
                    predsI[PAD_PART:, :PAD_START],
                    src[PAD_PART * SLOTS * 5:].rearrange("(p s c) -> p s c", p=1, c=5))

                wc = pim.tile([P, SLOTS], f32, tag="wc")
                hc = pim.tile([P, SLOTS], f32, tag="hc")
                drvI = pim.tile([P, SLOTS, 5], f32, tag="drvI")
                x1p = drvI[:, :, 0]
                x2p = drvI[:, :, 1]
                y1p = drvI[:, :, 2]
                y2p = drvI[:, :, 3]
                apred = drvI[:, :, 4]
                half = pim.tile([P, SLOTS], f32, tag="half")
                nc.vector.tensor_scalar_max(wc[:], predsI[:, :, 2], 1e-4)
                nc.vector.tensor_scalar_max(hc[:], predsI[:, :, 3], 1e-4)
                nc.vector.tensor_scalar_mul(half[:], wc[:], 0.5)
                nc.gpsimd.tensor_tensor(x1p, predsI[:, :, 0], half[:], op=Alu.subtract)
                nc.gpsimd.tensor_tensor(x2p, predsI[:, :, 0], half[:], op=Alu.add)
                nc.vector.tensor_scalar_mul(half[:], hc[:], 0.5)
                nc.gpsimd.tensor_tensor(y1p, predsI[:, :, 1], half[:], op=Alu.subtract)
                nc.gpsimd.tensor_tensor(y2p, predsI[:, :, 1], half[:], op=Alu.add)
                nc.gpsimd.tensor_tensor(apred, wc[:], hc[:], op=Alu.mult)
                nc.sync.dma_start(
                    drvI_d.ap()[b * P:(b + 1) * P]
                    .rearrange("p (s c) -> p s c", c=5), drvI[:])

                x1p6 = pim.tile([P, SLOTS], f16, tag="x1p6")
                x2p6 = pim.tile([P, SLOTS], f16, tag="x2p6")
                y1p6 = pim.tile([P, SLOTS], f16, tag="y1p6")
                y2p6 = pim.tile([P, SLOTS], f16, tag="y2p6")
                ap6 = pim.tile([P, SLOTS], f16, tag="ap6")
                for dst, s16 in ((x1p6, x1p), (x2p6, x2p), (y1p6, y1p),
                                 (y2p6, y2p), (ap6, apred)):
                    nc.scalar.copy(dst[:], s16)
                nc.scalar.copy(conf16[:, b], predsI[:, :, 4])
                trow = sml.tile([1, M, 4], f32, tag="trow")
                nc.sync.dma_start(trow[:], targets_d.ap()[b].unsqueeze(0))
                atrow = sml.tile([1, M, 2], f32, tag="atrow")
                nc.vector.tensor_sub(atrow[:, :, 0], trow[:, :, 2], trow[:, :, 0])
                nc.vector.tensor_sub(atrow[:, :, 1], trow[:, :, 3], trow[:, :, 1])
                nc.vector.tensor_tensor(atrow[:, :, 0], atrow[:, :, 0],
                                        atrow[:, :, 1], op=Alu.mult)
                x1tm = pim.tile([P, M, UG], f16, tag="x1tm")
                y1tm = pim.tile([P, M, UG], f16, tag="y1tm")
                x2tm = pim.tile([P, M, UG], f16, tag="x2tm")
                y2tm = pim.tile([P, M, UG], f16, tag="y2tm")
                for dst, rowap in ((x1tm, trow[:, :, 0]), (y1tm, trow[:, :, 1]),
                                   (x2tm, trow[:, :, 2]), (y2tm, trow[:, :, 3])):
                    pt = psum2.tile([P, M], f32, tag="bc_ps", name="bc_ps")
                    nc.tensor.matmul(pt[:], ones_row[:], rowap, start=True, stop=True)
                    nc.scalar.copy(dst[:], pt[:].unsqueeze(2).to_broadcast([P, M, UG]))
                # at row broadcast to [P, M] f32 (small; for the post-reduce scale)
                at_ps = psum2.tile([P, M], f32, tag="bc_ps", name="at_ps")
                nc.tensor.matmul(at_ps[:], ones_row[:], atrow[:, :, 0],
                                 start=True, stop=True)
                at64 = pim.tile([P, M], f32, tag="at64")
                nc.scalar.copy(at64[:], at_ps[:])

                imgs.append(dict(x1p6=x1p6, x2p6=x2p6, y1p6=y1p6, y2p6=y2p6,
                                 ap6=ap6, apredap=apred, x1tm=x1tm, y1tm=y1tm,
                                 x2tm=x2tm, y2tm=y2tm, at64=at64))

            tgj = ref.tile([P, 4], f32, tag="tgj")
            nc.sync.dma_start(tgj[:], targets_d.ap().rearrange("b m c -> (b m) c"))
            atj = ref.tile([P, 1], f32, tag="atj")
            a0 = ref.tile([P, 1], f32, tag="atj_a")
            nc.vector.tensor_sub(atj[:], tgj[:, 2:3], tgj[:, 0:1])
            nc.vector.tensor_sub(a0[:], tgj[:, 3:4], tgj[:, 1:2])
            nc.vector.tensor_tensor(atj[:], atj[:], a0[:], op=Alu.mult)

            # ========== focal bulk via ACT exp/ln (f16, both images) ==========
            # emitted early: fills DMA/startup dead time; table natural_log_exp
            FF = IMGS_PER_CORE * SLOTS
            xall = conf16[:].rearrange("p b s -> p (b s)")
            fb = lambda t, dt=f16: ref.tile([P, FF], dt, tag="fb" + t, name="fb" + t)
            ax_, e_, l_, sp_, e2_ = fb("ax"), fb("e"), fb("l"), fb("sp"), fb("e2")
            w2_, sgf, sqf = fb("w2", f32), fb("sg", f32), fb("sq", f32)
            f0 = fb("f0", f32)
            nc.scalar.activation(ax_[:], xall, Act.Abs)
            nc.scalar.activation(e_[:], ax_[:], Act.Exp, scale=-1.0)
            nc.vector.tensor_scalar_add(e_[:], e_[:], 1.0)
            nc.scalar.activation(l_[:], e_[:], Act.Ln)
            nc.vector.tensor_scalar_max(sp_[:], xall, 0.0)      # relu(x)
            nc.vector.tensor_add(sp_[:], sp_[:], l_[:])          # softplus
            nc.scalar.activation(e2_[:], xall, Act.Exp, scale=-1.0)
            nc.vector.tensor_scalar_add(w2_[:], e2_[:], 1.0)     # f32 out
            nc.vector.reciprocal(sgf[:], w2_[:])                 # sigmoid exact
            nc.vector.tensor_tensor(sqf[:], sgf[:], sgf[:], op=Alu.mult)
            nc.vector.tensor_tensor(f0[:], sqf[:], sp_[:], op=Alu.mult)
            frow2 = sml.tile([P, 2], f32, tag="frow2")
            f0v = f0[:].rearrange("p (b s) -> p b s", b=IMGS_PER_CORE)
            nc.vector.tensor_reduce(frow2[:, 0:1].unsqueeze(1), f0v[:, 0:1], axis=X,
                                    op=Alu.add)
            nc.vector.tensor_reduce(frow2[:, 1:2].unsqueeze(1), f0v[:, 1:2], axis=X,
                                    op=Alu.add)
            fsum_ps = psum.tile([2, 1], f32, tag="fsum_ps", name="fsum_ps")
            nc.tensor.matmul(fsum_ps[:], frow2[:], ones_col[:], start=True, stop=True)
            fsum = sml.tile([2, 1], f32, tag="fsum")
            nc.vector.tensor_copy(fsum[:], fsum_ps[:])
            nc.sync.dma_start(fsum_out_d.ap(), fsum[:])

            # ================= per-image coarse pass =================
            mall_ps = psum.tile([P, P], f32, tag="mall_ps", name="mall_ps")
            pall = ref.tile([P, K], u32, tag="pall")
            pallf = ref.tile([P, K], f32, tag="pallf")
            mx8 = ref.tile([P, 8], f32, tag="mx8")
            pi8 = ref.tile([P, 8], u32, tag="pi8")
            rowoff_f = ref.tile([P, K], f32, tag="rowoff_f")
            rowoff = ref.tile([P, K], u32, tag="rowoff")
            gall = ref.tile([P, K, SLOTS, 5], f32, tag="gall")
            for b in range(IMGS_PER_CORE):
                im = imgs[b]
                x1p6, x2p6, y1p6, y2p6 = (im["x1p6"], im["x2p6"], im["y1p6"],
                                          im["y2p6"])
                x1tm, y1tm, x2tm, y2tm = (im["x1tm"], im["y1tm"], im["x2tm"],
                                          im["y2tm"])
                macc = pim.tile([P, M, UG], f16, tag="macc")
                pend = []
                for g in range(NG):
                    s = slice(g * UG, (g + 1) * UG)

                    def pv(t):
                        return t[:, s].unsqueeze(1).to_broadcast([P, M, UG])

                    ltx = grp.tile([P, M, UG], f16, tag="ltx")
                    rbx = grp.tile([P, M, UG], f16, tag="rbx")
                    lty = grp.tile([P, M, UG], f16, tag="lty")
                    rby = grp.tile([P, M, UG], f16, tag="rby")

                    nc.vector.tensor_tensor(lty[:], pv(y1p6), y1tm[:], op=Alu.max)
                    nc.vector.tensor_tensor(rby[:], pv(y2p6), y2tm[:], op=Alu.min)
                    nc.vector.tensor_tensor(ltx[:], pv(x1p6), x1tm[:], op=Alu.max)
                    nc.vector.tensor_tensor(rbx[:], pv(x2p6), x2tm[:], op=Alu.min)
                    dxt = rbx
                    if g in (1, 3):
                        nc.gpsimd.tensor_tensor(dxt[:], rbx[:], ltx[:],
                                                op=Alu.subtract)
                    else:
                        nc.vector.tensor_sub(dxt[:], rbx[:], ltx[:])
                    nc.scalar.activation(dxt[:], dxt[:], Act.Relu)
                    dyt = rby
                    nc.vector.tensor_sub(dyt[:], rby[:], lty[:])
                    if len(pend) >= 2:
                        nc.vector.tensor_tensor(macc[:], macc[:], pend.pop(0)[:],
                                                op=Alu.max)
                    inter = macc if g == 0 else lty
                    # inter on Pool (deferred macc) except the last group, which
                    # sits on the drain path
                    if g == NG - 1:
                        nc.vector.tensor_tensor(inter[:], dxt[:], dyt[:], op=Alu.mult)
                    else:
                        nc.gpsimd.tensor_tensor(inter[:], dxt[:], dyt[:], op=Alu.mult)
                    if g > 0:
                        pend.append(inter)
                while pend:
                    nc.vector.tensor_tensor(macc[:], macc[:], pend.pop(0)[:],
                                            op=Alu.max)

                m1 = pim.tile([P, M], f32, tag="m1")
                # two-step reduce: f16 2x TT-max halves, then 14-wide reduce
                mh = pim.tile([P, M, UG // 2], f16, tag="mh")
                nc.vector.tensor_tensor(mh[:], macc[:, :, :UG // 2],
                                        macc[:, :, UG // 2:], op=Alu.max)
                nc.vector.tensor_reduce(m1[:], mh[:], axis=X, op=Alu.max)
                # per-partition exact normalization: rsc = 1/(ap_mid + at)
                apmid = sml.tile([P, 1], f32, tag="apmid")
                nc.vector.tensor_copy(apmid[:], im["apredap"][:, SLOTS // 2:SLOTS // 2 + 1])
                ssp = sml.tile([P, M], f32, tag="ssp")
                nc.vector.tensor_scalar(ssp[:], im["at64"][:], apmid[:], None,
                                        op0=Alu.add)
                nc.vector.reciprocal(ssp[:], ssp[:])
                nc.vector.tensor_tensor(m1[:], m1[:], ssp[:], op=Alu.mult)
                dbg(f"m1_{b}", m1[:], [P, M])
                nc.tensor.matmul(mall_ps[b * M:(b + 1) * M], m1[:], ident[:],
                                 start=True, stop=True)
            # ================= joint top-K =================
            nc.vector.tensor_copy(mall[:], mall_ps[:])
            dbg("mall", mall[:], [P, P])
            nc.vector.max(mx8[:], mall[:])
            nc.vector.max_index(pi8[:], mx8[:], mall[:])
            nc.vector.tensor_copy(pall[:], pi8[:, :K])
            nc.vector.tensor_copy(pallf[:], pi8[:, :K])  # u32 -> f32
            dbg("pall", pall[:], [P, K], u32)
            nc.vector.tensor_scalar(rowoff_f[:], pallf[:], imgoffPf[:], None,
                                    op0=Alu.add)
            nc.vector.tensor_copy(rowoff[:], rowoff_f[:])
            for k in range(K):
                nc.gpsimd.indirect_dma_start(
                    out=gall[:, k].rearrange("p s c -> p (s c)"), out_offset=None,
                    in_=drvI_d.ap(),
                    in_offset=IndirectOffsetOnAxis(ap=rowoff[:, k:k + 1], axis=0))
            qrow = ref.tile([P, K, SLOTS], f32, tag="qrow")
            rlt = ref.tile([P, K, SLOTS], f32, tag="rlt")
            rrb = ref.tile([P, K, SLOTS], f32, tag="rrb")
            rdx = ref.tile([P, K, SLOTS], f32, tag="rdx")
            rdy = ref.tile([P, K, SLOTS], f32, tag="rdy")
            # per-k refine: each k's compute overlaps the later gathers
            for k in range(K):
                gx1 = gall[:, k, :, 0]
                gx2 = gall[:, k, :, 1]
                gy1 = gall[:, k, :, 2]
                gy2 = gall[:, k, :, 3]
                gap = gall[:, k, :, 4]
                kl = lambda t: t[:, k]
                nc.vector.tensor_scalar(kl(rlt), gx1, tgj[:, 0:1], None, op0=Alu.max)
                nc.vector.tensor_scalar(kl(rrb), gx2, tgj[:, 2:3], None, op0=Alu.min)
                nc.vector.tensor_sub(kl(rdx), kl(rrb), kl(rlt))
                nc.vector.tensor_scalar_max(kl(rdx), kl(rdx), 0.0)
                nc.vector.tensor_scalar(kl(rlt), gy1, tgj[:, 1:2], None, op0=Alu.max)
                nc.vector.tensor_scalar(kl(rrb), gy2, tgj[:, 3:4], None, op0=Alu.min)
                nc.vector.tensor_sub(kl(rdy), kl(rrb), kl(rlt))
                nc.vector.tensor_tensor(kl(rlt), kl(rdx), kl(rdy), op=Alu.mult)
                nc.vector.tensor_scalar(kl(rrb), gap, atj[:], None, op0=Alu.add)
                nc.vector.reciprocal(kl(rdx), kl(rrb))
                nc.vector.tensor_tensor(kl(qrow), kl(rlt), kl(rdx), op=Alu.mult)
            dbg("qrow", qrow[:], [P, K, SLOTS])

            # full-row argmax over [P, K*SLOTS]
            qflat = qrow[:].rearrange("p k s -> p (k s)")
            kk8 = sml.tile([P, 8], f32, tag="kk8")
            ki8 = sml.tile([P, 8], u32, tag="ki8")
            nc.vector.max(kk8[:], qflat)
            nc.vector.max_index(ki8[:], kk8[:], qflat)
            kif = sml.tile([P, 1], f32, tag="kif")
            nc.vector.tensor_copy(kif[:], ki8[:, 0:1])
            selk = sml.tile([P, 1], f32, tag="selk")
            nc.vector.tensor_scalar(selk[:], kif[:], 0.5, float(1.0 / SLOTS),
                                    op0=Alu.add, op1=Alu.mult)
            # int copy ROUNDS-to-nearest; shift by -0.5 so round == floor
            nc.vector.tensor_scalar_add(selk[:], selk[:], -0.5)
            selki = sml.tile([P, 1], i32, tag="selki")
            nc.vector.tensor_copy(selki[:], selk[:])
            nc.vector.tensor_copy(selk[:], selki[:])     # back to f32
            selslot = sml.tile([P, 1], f32, tag="selslot")
            nc.vector.tensor_scalar(selslot[:], selk[:], float(-SLOTS), kif[:],
                                    op0=Alu.mult, op1=Alu.add)
            # select pall[selk]
            eqk = sml.tile([P, 8], f32, tag="eqk")
            nc.vector.tensor_scalar(eqk[:], iota_kf[:], selk[:], None, op0=Alu.is_equal)
            pallf8 = sml.tile([P, 8], f32, tag="pallf8")
            nc.gpsimd.memset(pallf8[:], 0.0)
            nc.vector.tensor_copy(pallf8[:, :K], pallf[:])
            tmpk = sml.tile([P, 8], f32, tag="tmpk")
            nc.vector.tensor_tensor(tmpk[:], pallf8[:], eqk[:], op=Alu.mult)
            selp = sml.tile([P, 1], f32, tag="selp")
            nc.vector.tensor_reduce(selp[:], tmpk[:], axis=X, op=Alu.add)
            nstar_f = sml.tile([P, 1], f32, tag="nstar_f")
            nc.vector.tensor_scalar(nstar_f[:], selp[:], float(SLOTS), selslot[:],
                                    op0=Alu.mult, op1=Alu.add)
            nc.vector.tensor_scalar_min(nstar_f[:], nstar_f[:], float(N - 1))
            dbg("ki8", ki8[:], [P, 8], u32)
            dbg("selk", selk[:], [P, 1])
            dbg("selslot", selslot[:], [P, 1])
            dbg("selp", selp[:], [P, 1])
            dbg("qrow2", qrow[:], [P, K, SLOTS])
            maxq = kk8[:, 0:1]
            thr = sml.tile([P, 1], f32, tag="thr")
            nc.vector.tensor_scalar(thr[:], maxq, float(1.0 / 6.0), None, op0=Alu.is_gt)
            dbg("nstar_f", nstar_f[:], [P, 1])
            dbg("thr", thr[:], [P, 1])
            dbg("maxq", kk8[:, 0:1], [P, 1])

            # ====== gather matched preds early (SWDGE latency overlaps dedup) ======
            g5 = sml.tile([P, 5], f32, tag="g5")
            nrow_f = sml.tile([P, 1], f32, tag="nrow_f")
            nc.vector.tensor_scalar(nrow_f[:], nstar_f[:], imgoffNf[:], None, op0=Alu.add)
            nrow = sml.tile([P, 1], u32, tag="nrow")
            nc.vector.tensor_copy(nrow[:], nrow_f[:])
            nc.gpsimd.indirect_dma_start(
                out=g5[:], out_offset=None,
                in_=preds_d.ap().rearrange("b n c -> (b n) c"),
                in_offset=IndirectOffsetOnAxis(ap=nrow[:], axis=0))
            dbg("g5", g5[:], [P, 5])

            # ================= dedup (block-masked, DVE-only) =================
            pair = sml.tile([P, 2], f32, tag="pair")
            nc.vector.tensor_copy(pair[:, 0:1], nstar_f[:])
            nc.vector.tensor_copy(pair[:, 1:2], thr[:])
            pairT_ps = psum.tile([1, 2, P], f32, tag="pairT_ps", name="pairT_ps")
            nc.tensor.transpose(pairT_ps[:, 0], pair[:, 0:1], ident[:])
            nc.tensor.transpose(pairT_ps[:, 1], pair[:, 1:2], ident[:])
            pairT = sml.tile([1, 2, P], f32, tag="pairT")
            nc.vector.tensor_copy(pairT[:], pairT_ps[:])
            rowB_ps = psum.tile([P, 2, P], f32, tag="rowB_ps", name="rowB_ps")
            nc.tensor.matmul(rowB_ps[:].rearrange("p a b -> p (a b)"), ones_row[:],
                             pairT[:].rearrange("o a b -> o (a b)"),
                             start=True, stop=True)
            rowB = ref.tile([P, 2, P], f32, tag="rowB")
            nc.vector.tensor_copy(rowB[:], rowB_ps[:])
            eq = ref.tile([P, P], f32, tag="eq")
            nc.vector.tensor_scalar(eq[:], rowB[:, 0], nstar_f[:], None,
                                    op0=Alu.is_equal)
            nc.vector.tensor_tensor(eq[:], eq[:], rowB[:, 1], op=Alu.mult)
            nc.vector.tensor_tensor(eq[:], eq[:], ltmask[:], op=Alu.mult)
            blocked = sml.tile([P, 1], f32, tag="blocked")
            nc.vector.tensor_reduce(blocked[:], eq[:], axis=X, op=Alu.max)
            ok = sml.tile([P, 1], f32, tag="ok")
            nc.vector.tensor_scalar(ok[:], blocked[:], -1.0, 1.0,
                                    op0=Alu.mult, op1=Alu.add)
            nc.vector.tensor_tensor(ok[:], ok[:], thr[:], op=Alu.mult)
            dbg("ok", ok[:], [P, 1])

            # ================= ciou, x/y lanes packed as [P, 2] =================
            t1 = lambda tag: sml.tile([P, 1], f32, tag=tag, name=tag)
            t2l = lambda tag: sml.tile([P, 2], f32, tag=tag, name=tag)
            half2, p1, p2 = t2l("half2"), t2l("p1"), t2l("p2")
            nc.vector.tensor_scalar(half2[:], g5[:, 2:4], 1e-4, 0.5,
                                    op0=Alu.max, op1=Alu.mult)
            nc.vector.tensor_sub(p1[:], g5[:, 0:2], half2[:])
            nc.vector.tensor_add(p2[:], g5[:, 0:2], half2[:])
            t1c, t2c = tgj[:, 0:2], tgj[:, 2:4]

            lt2, rb2, d2 = t2l("lt2"), t2l("rb2"), t2l("d2")
            nc.vector.tensor_tensor(lt2[:], p1[:], t1c, op=Alu.max)
            nc.vector.tensor_tensor(rb2[:], p2[:], t2c, op=Alu.min)
            nc.vector.tensor_sub(d2[:], rb2[:], lt2[:])
            nc.vector.tensor_scalar_max(d2[:], d2[:], 0.0)
            ginter = t1("ginter")
            nc.vector.tensor_tensor(ginter[:], d2[:, 0:1], d2[:, 1:2], op=Alu.mult)
            whp, wht = t2l("whp"), t2l("wht")
            nc.vector.tensor_sub(whp[:], p2[:], p1[:])
            nc.vector.tensor_sub(wht[:], t2c, t1c)
            gu = t1("gu")
            a1 = t1("a1")
            nc.vector.tensor_tensor(gu[:], whp[:, 0:1], whp[:, 1:2], op=Alu.mult)
            nc.vector.tensor_tensor(a1[:], wht[:, 0:1], wht[:, 1:2], op=Alu.mult)
            nc.vector.tensor_add(gu[:], gu[:], a1[:])
            nc.vector.tensor_sub(gu[:], gu[:], ginter[:])
            giou = t1("giou")
            nc.vector.tensor_scalar_add(gu[:], gu[:], float(EPS))
            nc.vector.reciprocal(gu[:], gu[:])
            nc.vector.tensor_tensor(giou[:], ginter[:], gu[:], op=Alu.mult)
            # smallest enclosing box diag + center distance, lanes packed
            c1, c2 = t2l("c1"), t2l("c2")
            nc.vector.tensor_tensor(c1[:], p1[:], t1c, op=Alu.min)
            nc.vector.tensor_tensor(c2[:], p2[:], t2c, op=Alu.max)
            nc.vector.tensor_sub(c2[:], c2[:], c1[:])
            nc.vector.tensor_tensor(c2[:], c2[:], c2[:], op=Alu.mult)
            diag = t1("diag")
            nc.vector.tensor_add(diag[:], c2[:, 0:1], c2[:, 1:2])
            nc.vector.tensor_scalar_add(diag[:], diag[:], float(EPS))
            s12 = t2l("s12")
            nc.vector.tensor_add(s12[:], p1[:], p2[:])
            nc.vector.tensor_sub(s12[:], s12[:], t1c)
            nc.vector.tensor_sub(s12[:], s12[:], t2c)
            nc.vector.tensor_tensor(s12[:], s12[:], s12[:], op=Alu.mult)
            cent = t1("cent")
            nc.vector.tensor_add(cent[:], s12[:, 0:1], s12[:, 1:2])
            nc.vector.tensor_scalar_mul(cent[:], cent[:], 0.25)
            diou = t1("diou")
            nc.vector.reciprocal(diag[:], diag[:])
            nc.vector.tensor_tensor(diou[:], cent[:], diag[:], op=Alu.mult)
            nc.vector.tensor_sub(diou[:], diou[:], giou[:])
            nc.vector.tensor_scalar_add(diou[:], diou[:], 1.0)
            vv = t1("vv")
            rat = sml.tile([P, 2], f32, tag="rat", name="rat")
            big2 = sml.tile([P, 2], i32, tag="big2", name="big2")
            inv2 = sml.tile([P, 2], f32, tag="inv2", name="inv2")
            s2 = sml.tile([P, 2], f32, tag="s2", name="s2")
            ac2 = sml.tile([P, 2], f32, tag="ac2", name="ac2")
            hh2 = sml.tile([P, 2], f32, tag="hh2", name="hh2")
            # rat = [wt/ht, wp/hp]
            nc.vector.tensor_copy(hh2[:, 0:1], wht[:, 1:2])
            nc.vector.tensor_copy(hh2[:, 1:2], whp[:, 1:2])
            nc.vector.reciprocal(hh2[:], hh2[:])
            nc.vector.tensor_copy(rat[:, 0:1], wht[:, 0:1])
            nc.vector.tensor_copy(rat[:, 1:2], whp[:, 0:1])
            nc.vector.tensor_tensor(rat[:], rat[:], hh2[:], op=Alu.mult)
            nc.vector.tensor_scalar(big2[:], rat[:], 1.0, None, op0=Alu.is_gt)
            nc.vector.reciprocal(inv2[:], rat[:])
            nc.vector.copy_predicated(rat[:], big2[:], inv2[:])
            nc.vector.tensor_tensor(s2[:], rat[:], rat[:], op=Alu.mult)
            nc.vector.tensor_scalar(ac2[:], s2[:], float(AT_POLY[0]),
                                    float(AT_POLY[1]), op0=Alu.mult, op1=Alu.add)
            for coef in AT_POLY[2:]:
                nc.vector.tensor_tensor(ac2[:], ac2[:], s2[:], op=Alu.mult)
                nc.vector.tensor_scalar_add(ac2[:], ac2[:], float(coef))
            nc.vector.tensor_tensor(ac2[:], ac2[:], rat[:], op=Alu.mult)
            nc.vector.tensor_scalar(inv2[:], ac2[:], -1.0, float(np.pi / 2),
                                    op0=Alu.mult, op1=Alu.add)
            nc.vector.copy_predicated(ac2[:], big2[:], inv2[:])
            nc.vector.tensor_sub(vv[:], ac2[:, 0:1], ac2[:, 1:2])
            nc.vector.tensor_tensor(vv[:], vv[:], vv[:], op=Alu.mult)
            nc.vector.tensor_scalar_mul(vv[:], vv[:], float(C_4PI2))
            nc.vector.tensor_scalar(a1[:], giou[:], -1.0, float(1.0 + EPS),
                                    op0=Alu.mult, op1=Alu.add)
            nc.vector.tensor_add(a1[:], a1[:], vv[:])
            nc.vector.reciprocal(a1[:], a1[:])
            nc.vector.tensor_tensor(a1[:], a1[:], vv[:], op=Alu.mult)
            ciou = t1("ciou")
            nc.vector.tensor_tensor(ciou[:], a1[:], vv[:], op=Alu.mult)
            nc.vector.tensor_add(ciou[:], ciou[:], diou[:])
            dbg("ciou", ciou[:], [P, 1])

            # ===== matched focal correction (f32 poly, no act tables) =====
            xm = g5[:, 4:5]

            def softplus_sigmoid32(x_ap, shape, pfx):
                tl = lambda t: sml.tile(shape, f32, tag=pfx + t, name=pfx + t)
                sg_, sp_2, u_, w_, z_, e_2 = (tl("sg"), tl("sp"), tl("u"),
                                              tl("w"), tl("z"), tl("e"))
                nc.scalar.activation(e_2[:], x_ap, Act.Exp, scale=-1.0)
                nc.vector.tensor_scalar_add(e_2[:], e_2[:], 1.0)
                nc.vector.reciprocal(sg_[:], e_2[:])
                nc.vector.tensor_scalar_mul(u_[:], x_ap, -1.0)
                nc.vector.tensor_tensor(u_[:], u_[:], x_ap, op=Alu.max)
                nc.scalar.activation(u_[:], u_[:], Act.Exp, scale=-1.0)
                nc.vector.tensor_scalar_add(w_[:], u_[:], 1.0)
                nc.scalar.activation(w_[:], w_[:], Act.Ln)
                nc.vector.tensor_scalar(z_[:], x_ap, 1.0, 0.0, op0=Alu.mult, op1=Alu.max)
                nc.vector.tensor_add(sp_2[:], z_[:], w_[:])
                return sg_, sp_2

            msg, msp = softplus_sigmoid32(xm, [P, 1], "fm")
            msn = t1("msn")
            nc.vector.tensor_sub(msn[:], msp[:], xm)
            mf0, mf1 = t1("mf0"), t1("mf1")
            nc.vector.tensor_tensor(mf0[:], msg[:], msg[:], op=Alu.mult)
            nc.vector.tensor_tensor(mf0[:], mf0[:], msp[:], op=Alu.mult)
            nc.vector.tensor_scalar_mul(mf0[:], mf0[:], 0.75)
            nc.vector.tensor_scalar(mf1[:], msg[:], -1.0, 1.0,
                                    op0=Alu.mult, op1=Alu.add)
            nc.vector.tensor_tensor(mf1[:], mf1[:], mf1[:], op=Alu.mult)
            nc.vector.tensor_tensor(mf1[:], mf1[:], msn[:], op=Alu.mult)
            nc.vector.tensor_scalar_mul(mf1[:], mf1[:], 0.25)
            nc.vector.tensor_sub(mf1[:], mf1[:], mf0[:])
            nc.vector.tensor_tensor(mf1[:], mf1[:], ok[:], op=Alu.mult)

            # ================= per-image sums via PE =================
            rhs3 = sml.tile([P, 3], f32, tag="rhs3")
            nc.vector.tensor_copy(rhs3[:, 0:1], mf1[:])
            nc.vector.tensor_tensor(rhs3[:, 1:2], ciou[:], ok[:], op=Alu.mult)
            nc.vector.tensor_copy(rhs3[:, 2:3], ok[:])
            sums_ps = psum.tile([2, 3], f32, tag="sums_ps", name="sums_ps")
            nc.tensor.matmul(sums_ps[:], halfind[:], rhs3[:], start=True, stop=True)
            sums = sml.tile([2, 3], f32, tag="sums")
            nc.vector.tensor_copy(sums[:], sums_ps[:])
            dbg("sums", sums[:], [2, 3])

            nc.sync.dma_start(sums_out_d.ap(), sums[:])

            lp.__exit__(None, None, None)

    nc.compile()
    return nc


def _get_nc():
    if "nc" not in _cache:
        _cache["nc"] = _build()
    return _cache["nc"]


def kernel(preds: np.ndarray, targets: np.ndarray) -> np.ndarray:
    from concourse.bass_utils import run_bass_kernel_spmd

    nc = _get_nc()
    preds = np.ascontiguousarray(preds, dtype=np.float32)
    targets = np.ascontiguousarray(targets, dtype=np.float32)
    # loss is permutation-invariant over preds; sort each image by box
    # area so every SBUF partition row holds ~equal-area preds (enables
    # the per-partition coarse normalization)
    area = (np.maximum(preds[:, :, 2], 1e-4) * np.maximum(preds[:, :, 3], 1e-4))
    order = np.argsort(area, axis=1, kind="stable")
    preds = np.take_along_axis(preds, order[:, :, None], axis=1)
    in_maps = []
    for c in range(N_CORES):
        s = c * IMGS_PER_CORE
        in_maps.append({"preds": preds[s:s + IMGS_PER_CORE],
                        "targets": targets[s:s + IMGS_PER_CORE]})
    res = run_bass_kernel_spmd(nc, in_maps, list(range(N_CORES)))
    per_image = []
    for c in range(N_CORES):
        s = np.asarray(res.results[c]["sums_out"], np.float64)   # [2,3] dsum,bsum,nm
        f = np.asarray(res.results[c]["fsum_out"], np.float64)[:, 0]
        per_image.append((0.75 * f + s[:, 0]) / N + s[:, 1] / np.maximum(s[:, 2], 1.0))
    return np.float32(np.concatenate(per_image).mean())
